# revision 11
# baseline (speedup 1.0000x reference)
"""Trainium2 Bass kernel for MultiHeadFrequencyCrossAttention.

Math note: the reference computes, per (batch, head) slice,
    energy = ifft2( fft2(Q) @ fft2(K)^T * dk ).real
Because the DFT matrix F satisfies F @ F^T = n * P (P = index-negation
permutation), this collapses EXACTLY to
    energy = dk * D * Q @ K~^T        with K~[j, d] = K[j, (-d) mod D]
i.e. plain attention with K's head-dim index flipped (mod D) and an extra
scale of dk * D = 512.  No FFTs are needed; the flip and scale are folded
into host-side slices of the Wk / Wq projection weights.

Sharding: 8 cores = 4 batches x 2 head-groups (4 heads each).  Each core
gets q[b]^T, kv[b]^T (pre-transposed on host so the contraction dim lands
on SBUF partitions) plus its slice of the projection weights, computes
attention for its 4 heads, and applies its slice of Wo.  The host sums the
two partial Wo products per batch (the unshard-reduce).

Per-core pipeline (T=1024 time steps, D=64 head dim):
  1. qp^T / kp~^T projections (per head, (64, 1024), fp32), kp~ gets an
     extra all-ones row; vp natural (t-major) with an extra all-ones column
     per head (bf16).
  2. max pass: S = qp^T.T @ kp~^T per 128-row block, DVE reduce_max
     (negated), assembled into an extra "-rowmax" row of qp^T via a tiny
     DRAM bounce.
  3. main pass: S^T - max via one K=65 matmul per 128-col block (the ones
     row of kp~^T times the -max row of qp^T injects the bias), ACT exp
     psum->sbuf bf16 directly in (j, i) layout.
  4. AV: A^T tiles are the matmul moving operand; vp (with ones column) is
     stationary, accumulating [Y^T; rowsums] in one psum tensor.
  5. normalize Y^T by 1/rowsums (partition_broadcast + DVE mul), then the
     Wo partial product, DMA out.
"""

import os
import numpy as np
from contextlib import ExitStack

import concourse.bass as bass
import concourse.tile as tile
from concourse import bacc, mybir
from concourse.bass_utils import run_bass_kernel_spmd

F32 = mybir.dt.float32
BF16 = mybir.dt.bfloat16
AX = mybir.AxisListType
AF = mybir.ActivationFunctionType

T = 1024          # sequence length
E = 512           # embed dim
H = 8             # total heads
D = E // H        # head dim = 64
NH = 4            # heads per core
DX = NH * (D + 1) # vp columns incl. ones = 260
N_CORES = 8
SCALE = float(D) * float(D) ** 0.5  # dk * D = 512.0

TRACE = False          # set by test harness; adds NTFF profiling
LAST_EXEC_NS = None


def _emit(ctx, tc, qT_d, kvT_d, wq_d, wk_d, wv_d, wo_d, out_d):
    nc = tc.nc
    const = ctx.enter_context(tc.tile_pool(name="const", bufs=1))
    ps_big = ctx.enter_context(tc.tile_pool(name="ps_big", bufs=2, space="PSUM"))
    ps_av = ctx.enter_context(tc.tile_pool(name="ps_av", bufs=1, space="PSUM"))
    ps_sm = ctx.enter_context(tc.tile_pool(name="ps_sm", bufs=2, space="PSUM"))
    atp = ctx.enter_context(tc.tile_pool(name="atp", bufs=4))
    outp = ctx.enter_context(tc.tile_pool(name="outp", bufs=3))
    dramp = ctx.enter_context(tc.tile_pool(name="dramp", bufs=1, space="DRAM"))

    # ---- input loads ----
    qT = [const.tile([128, T], F32, tag=f"qT{e}", name=f"qT{e}") for e in range(4)]
    kvT = [const.tile([128, T], F32, tag=f"kvT{e}", name=f"kvT{e}") for e in range(4)]
    wq = [const.tile([128, NH * D], F32, tag=f"wq{e}", name=f"wq{e}") for e in range(4)]
    wk = [const.tile([128, NH * D], F32, tag=f"wk{e}", name=f"wk{e}") for e in range(4)]
    wv = [const.tile([128, DX], F32, tag=f"wv{e}", name=f"wv{e}") for e in range(4)]
    wo = [const.tile([128, E], F32, tag=f"wo{g}", name=f"wo{g}") for g in range(2)]
    for e in range(4):
        sl = slice(e * 128, (e + 1) * 128)
        nc.sync.dma_start(qT[e][:], qT_d[sl, :])
        nc.sync.dma_start(kvT[e][:], kvT_d[sl, :])
        nc.sync.dma_start(wq[e][:], wq_d[sl, :])
        nc.sync.dma_start(wk[e][:], wk_d[sl, :])
        nc.sync.dma_start(wv[e][:], wv_d[sl, :])
    for g in range(2):
        nc.sync.dma_start(wo[g][:], wo_d[g * 128:(g + 1) * 128, :])

    # ---- projections ----
    # qpx[h]: rows 0:64 = qp^T (scaled), row 64 = -rowmax (filled later)
    # kpx[h]: rows 0:64 = kp~^T,        row 64 = ones
    qpx = [const.tile([65, T], F32, tag=f"qpx{h}", name=f"qpx{h}") for h in range(NH)]
    kpx = [const.tile([65, T], F32, tag=f"kpx{h}", name=f"kpx{h}") for h in range(NH)]
    for wt, src, dstx in ((wq, qT, qpx), (wk, kvT, kpx)):
        for h in range(NH):
            ps = ps_big.tile([64, T], F32, tag="big", name="psb")
            for n in range(2):
                nsl = slice(n * 512, (n + 1) * 512)
                for e in range(4):
                    nc.tensor.matmul(
                        ps[:, nsl],
                        lhsT=wt[e][:, h * D:(h + 1) * D],
                        rhs=src[e][:, nsl],
                        start=(e == 0), stop=(e == 3),
                    )
            nc.scalar.copy(dstx[h][0:64, :], ps[:])
    for h in range(NH):
        nc.vector.memset(kpx[h][64:65, :], 1.0)

    # vp natural (t-major) + ones columns, bf16
    vpx = [const.tile([128, DX], BF16, tag=f"vpx{t}", name=f"vpx{t}") for t in range(8)]
    for t in range(8):
        ps = ps_sm.tile([128, E], F32, tag="sm", name="pss")
        for e in range(4):
            nc.tensor.matmul(
                ps[:, 0:DX],
                lhsT=kvT[e][:, t * 128:(t + 1) * 128],
                rhs=wv[e][:],
                start=(e == 0), stop=(e == 3),
            )
        nc.vector.tensor_copy(vpx[t][:], ps[:, 0:DX])
        for h4 in range(NH):
            c = h4 * (D + 1) + D
            nc.gpsimd.memset(vpx[t][:, c:c + 1], 1.0)

    # ---- per-head attention ----
    ypk = [const.tile([128, T], F32, tag=f"ypk{g}", name=f"ypk{g}") for g in range(2)]
    for h in range(NH):
        # max pass: S natural, row max per 128-row block
        colmax = const.tile([128, 8], F32, tag=f"cm{h}", name=f"cm{h}")
        for i in range(8):
            ps = ps_big.tile([128, T], F32, tag="big", name="psb")
            for n in range(2):
                nsl = slice(n * 512, (n + 1) * 512)
                nc.tensor.matmul(
                    ps[:, nsl],
                    lhsT=qpx[h][0:64, i * 128:(i + 1) * 128],
                    rhs=kpx[h][0:64, nsl],
                    start=True, stop=True,
                )
            nc.vector.reduce_max(colmax[:, i:i + 1], ps[:], axis=AX.X,
                                 negate=True)
        # (128, 8) -> (1, 1024) row of qpx via DRAM bounce
        sc = dramp.tile([8, 128], F32, tag=f"sc{h}", name=f"sc{h}")
        nc.sync.dma_start(sc[:].rearrange("c p -> p c"), colmax[:])
        nc.sync.dma_start(qpx[h][64:65, :], sc[:].rearrange("c p -> (c p)"))

        # main pass: S^T - max, exp, AV accumulate
        oex = ps_av.tile([65, T], F32, tag="av", name="oex")
        for j in range(8):
            ps = ps_big.tile([128, T], F32, tag="big", name="psb")
            for n in range(2):
                nsl = slice(n * 512, (n + 1) * 512)
                nc.tensor.matmul(
                    ps[:, nsl],
                    lhsT=kpx[h][:, j * 128:(j + 1) * 128],
                    rhs=qpx[h][:, nsl],
                    start=True, stop=True,
                )
            at = atp.tile([128, T], BF16, tag="at", name="at")
            nc.scalar.activation(at[:], ps[:], AF.Exp)
            for n in range(2):
                nsl = slice(n * 512, (n + 1) * 512)
                nc.tensor.matmul(
                    oex[:, nsl],
                    lhsT=vpx[j][:, h * (D + 1):(h + 1) * (D + 1)],
                    rhs=at[:, nsl],
                    start=(j == 0), stop=(j == 7),
                )

        # normalize: Y^T = oex[0:64] / rowsums (oex row 64)
        recip = const.tile([1, T], F32, tag=f"rcp{h}", name=f"rcp{h}")
        nc.vector.reciprocal(recip[:], oex[64:65, :])
        recb = const.tile([64, T], F32, tag=f"rcb{h}", name=f"rcb{h}")
        nc.gpsimd.partition_broadcast(recb[:], recip[:])
        g, half = divmod(h, 2)
        nc.vector.tensor_mul(
            ypk[g][half * 64:(half + 1) * 64, :], oex[0:64, :], recb[:]
        )

    # ---- output projection ----
    for i in range(8):
        pso = ps_sm.tile([128, E], F32, tag="sm", name="pso")
        for g in range(2):
            nc.tensor.matmul(
                pso[:],
                lhsT=ypk[g][:, i * 128:(i + 1) * 128],
                rhs=wo[g][:],
                start=(g == 0), stop=(g == 1),
            )
        ot = outp.tile([128, E], F32, tag="ot", name="ot")
        nc.vector.tensor_copy(ot[:], pso[:])
        nc.sync.dma_start(out_d[i * 128:(i + 1) * 128, :], ot[:])


def build_program():
    # Bacc (not raw Bass): its compile() splits multi-sem matmul waits onto
    # ldweights (TRN2 allows 1 wait/instruction), auto-inserts gpsimd
    # library loads for PartitionBroadcast, and lowers extended-ISA bytes.
    nc = bacc.Bacc("TRN2", target_bir_lowering=False, debug=False)
    dp = nc.declare_dram_parameter
    qT_d = dp("qT", [E, T], F32, isOutput=False)
    kvT_d = dp("kvT", [E, T], F32, isOutput=False)
    wq_d = dp("wq", [E, NH * D], F32, isOutput=False)
    wk_d = dp("wk", [E, NH * D], F32, isOutput=False)
    wv_d = dp("wv", [E, DX], F32, isOutput=False)
    wo_d = dp("wo", [NH * D, E], F32, isOutput=False)
    out_d = dp("out", [T, E], F32, isOutput=True)
    with ExitStack() as ctx:
        tc = ctx.enter_context(tile.TileContext(nc))
        _emit(ctx, tc, qT_d, kvT_d, wq_d, wk_d, wv_d, wo_d, out_d)
    nc.finalize()  # Bacc.finalize runs compile() then freezes
    return nc


_PROGRAM = None


def _get_program():
    global _PROGRAM
    if _PROGRAM is None:
        _PROGRAM = build_program()
    return _PROGRAM


def make_in_maps(q, kv, Wq, Wk, Wv, Wo):
    in_maps = []
    for c in range(N_CORES):
        b, g = divmod(c, 2)
        heads = [g * NH + j for j in range(NH)]
        idx_q = [d * H + h for h in heads for d in range(D)]
        idx_k = [((D - d) % D) * H + h for h in heads for d in range(D)]
        wv_c = np.zeros((E, DX), np.float32)
        for j, h in enumerate(heads):
            wv_c[:, j * (D + 1):j * (D + 1) + D] = Wv[:, [d * H + h for d in range(D)]]
        in_maps.append({
            "qT": np.ascontiguousarray(q[b].T),
            "kvT": np.ascontiguousarray(kv[b].T),
            "wq": np.ascontiguousarray(Wq[:, idx_q]) * np.float32(SCALE),
            "wk": np.ascontiguousarray(Wk[:, idx_k]),
            "wv": wv_c,
            "wo": np.ascontiguousarray(Wo[g * NH * D:(g + 1) * NH * D, :]),
        })
    return in_maps


def kernel(**inputs):
    global LAST_EXEC_NS
    q = np.asarray(inputs["q"], dtype=np.float32)
    kv = np.asarray(inputs["kv"], dtype=np.float32)
    Wq = np.asarray(inputs["Wq"], dtype=np.float32)
    Wk = np.asarray(inputs["Wk"], dtype=np.float32)
    Wv = np.asarray(inputs["Wv"], dtype=np.float32)
    Wo = np.asarray(inputs["Wo"], dtype=np.float32)
    B = q.shape[0]

    nc = _get_program()
    in_maps = make_in_maps(q, kv, Wq, Wk, Wv, Wo)
    res = run_bass_kernel_spmd(nc, in_maps, list(range(N_CORES)), trace=TRACE)
    LAST_EXEC_NS = res.exec_time_ns

    out = np.empty((B, T, E), np.float32)
    for b in range(B):
        out[b] = res.results[2 * b]["out"] + res.results[2 * b + 1]["out"]
    return out


# revision 12
# speedup vs baseline: 1.6485x; 1.6485x over previous
"""Trainium2 Bass kernel for MultiHeadFrequencyCrossAttention.

Math note: the reference computes, per (batch, head) slice,
    energy = ifft2( fft2(Q) @ fft2(K)^T * dk ).real
Because the DFT matrix F satisfies F @ F^T = n * P (P = index-negation
permutation), this collapses EXACTLY to
    energy = dk * D * Q @ K~^T        with K~[j, d] = K[j, (-d) mod D]
i.e. plain attention with K's head-dim index flipped (mod D) and an extra
scale of dk * D = 512.  No FFTs are needed; the flip and scale are folded
into host-side slices of the Wk / Wq projection weights.

Sharding: 8 cores = 4 batches x 2 head-groups (4 heads each).  Each core
gets q[b]^T, kv[b]^T (pre-transposed on host so the contraction dim lands
on SBUF partitions) plus its slice of the projection weights, computes
attention for its 4 heads, and applies its slice of Wo.  The host sums the
two partial Wo products per batch (the unshard-reduce).

Precision scheme (PE fp32 matmuls are 4 cyc/row; fp16 is 1 cyc/row):
every value on the logit path is split hi/lo into two fp16 parts
(x = xh + xl, products of fp16 are exact in the fp32 PSUM accumulator), so
  x @ y ~= xh@yh + (xh@yl + xl@yh)     [~22-bit mantissa, err ~1e-6 rel]
One extra all-ones row in the stationary K operand times a "-rowmax" row
in the moving Q operand injects the softmax max-subtraction bias directly
into the S^T matmul.  The row max itself comes from a separate hi-only
fp16 pass (error ~ +-15 absolute on ~25000-scale logits, well inside the
exp() range window since A tiles are bf16).  A/V/output paths are plain
16-bit (error there stays relative, ~2e-3, no sharp-softmax blowup).

Per-core pipeline (T=1024, D=64):
  1. hi/lo projections -> per head: qm/km (fp16 hi + bias/ones row),
     qc/kc (fp16 [lo;hi] stacks for the cross matmul); vp t-major bf16
     with an all-ones column per head.
  2. max pass: S = qh @ kh^T per 128-row block (fp16), DVE reduce_max
     (negated) -> DRAM bounce -> fp16 "-rowmax" row of qm.
  3. main pass: S^T - max = cross(K=128) + main(K=65, w/ bias row)
     matmuls, ACT exp psum->sbuf bf16 directly in (j, i) layout.
  4. AV: A^T tiles are the moving operand; vp (with ones column) is
     stationary, accumulating [Y^T; rowsums] in one psum tensor.
  5. normalize Y^T by 1/rowsums (partition_broadcast + DVE mul), cast
     fp16, Wo partial product (fp16), DMA out.
"""

import numpy as np
from contextlib import ExitStack

import concourse.bass as bass
import concourse.tile as tile
from concourse import bacc, mybir
from concourse.bass_utils import run_bass_kernel_spmd

F32 = mybir.dt.float32
F16 = mybir.dt.float16
BF16 = mybir.dt.bfloat16
AX = mybir.AxisListType
AF = mybir.ActivationFunctionType

T = 1024          # sequence length
E = 512           # embed dim
H = 8             # total heads
D = E // H        # head dim = 64
NH = 4            # heads per core
DX = NH * (D + 1) # vp columns incl. ones = 260
N_CORES = 8
SCALE = float(D) * float(D) ** 0.5  # dk * D = 512.0

TRACE = False          # set by test harness; adds NTFF profiling
LAST_EXEC_NS = None


def _emit(ctx, tc, dram):
    nc = tc.nc
    const = ctx.enter_context(tc.tile_pool(name="const", bufs=1))
    ps_big = ctx.enter_context(tc.tile_pool(name="ps_big", bufs=2, space="PSUM"))
    ps_av = ctx.enter_context(tc.tile_pool(name="ps_av", bufs=1, space="PSUM"))
    ps_sm = ctx.enter_context(tc.tile_pool(name="ps_sm", bufs=2, space="PSUM"))
    atp = ctx.enter_context(tc.tile_pool(name="atp", bufs=4))
    outp = ctx.enter_context(tc.tile_pool(name="outp", bufs=3))
    dramp = ctx.enter_context(tc.tile_pool(name="dramp", bufs=1, space="DRAM"))

    # ---- input loads (all fp16 on the wire) ----
    def load4(name, cols):
        ts = [const.tile([128, cols], F16, tag=f"{name}{e}", name=f"{name}{e}")
              for e in range(4)]
        for e in range(4):
            nc.sync.dma_start(ts[e][:], dram[name][e * 128:(e + 1) * 128, :])
        return ts

    qh_in = load4("qh", T)    # q^T hi / lo
    ql_in = load4("ql", T)
    kvh_in = load4("kvh", T)  # kv^T hi / lo
    kvl_in = load4("kvl", T)
    wqh = load4("wqh", NH * D)
    wql = load4("wql", NH * D)
    wkh = load4("wkh", NH * D)
    wkl = load4("wkl", NH * D)
    wv = load4("wv", DX)
    wo = [const.tile([128, E], F16, tag=f"wo{g}", name=f"wo{g}") for g in range(2)]
    for g in range(2):
        nc.sync.dma_start(wo[g][:], dram["wo"][g * 128:(g + 1) * 128, :])

    # ---- hi/lo projections ----
    # per head: qm (65, T) fp16 = [qp_hi; -rowmax(fp16) later]
    #           km (65, T) fp16 = [kp_hi; ones]
    #           qc (128, T) fp16 = [qp_lo; qp_hi]   (cross moving operand)
    #           kc (128, T) fp16 = [kp_hi; kp_lo]   (cross stationary)
    qm = [const.tile([65, T], F16, tag=f"qm{h}", name=f"qm{h}") for h in range(NH)]
    km = [const.tile([65, T], F16, tag=f"km{h}", name=f"km{h}") for h in range(NH)]
    qc = [const.tile([128, T], F16, tag=f"qc{h}", name=f"qc{h}") for h in range(NH)]
    kc = [const.tile([128, T], F16, tag=f"kc{h}", name=f"kc{h}") for h in range(NH)]

    for wh, wl, xh, xl, dm, dc, hi_row in (
        (wqh, wql, qh_in, ql_in, qm, qc, 64),   # qc rows: [lo; hi]
        (wkh, wkl, kvh_in, kvl_in, km, kc, 0),  # kc rows: [hi; lo]
    ):
        for m in range(2):  # head pair
            msl = slice(m * 128, (m + 1) * 128)
            ps = ps_big.tile([128, T], F32, tag="big", name="psb")
            for n in range(2):
                nsl = slice(n * 512, (n + 1) * 512)
                mms = (
                    # cross: Wh @ xl  +  Wl @ xh
                    [(wh[e], xl[e]) for e in range(4)]
                    + [(wl[e], xh[e]) for e in range(4)]
                    # main: Wh @ xh
                    + [(wh[e], xh[e]) for e in range(4)]
                )
                for i_mm, (lw, rx) in enumerate(mms):
                    nc.tensor.matmul(
                        ps[:, nsl],
                        lhsT=lw[:, msl],
                        rhs=rx[:, nsl],
                        start=(i_mm == 0), stop=(i_mm == len(mms) - 1),
                    )
            for hh in range(2):
                h = 2 * m + hh
                psl = slice(hh * 64, hh * 64 + 64)
                lo_row = 64 - hi_row
                # hi part (fp16 cast) into the K=65 "main" tile
                nc.scalar.copy(dm[h][0:64, :], ps[psl, :])
                # hi copy into the cross tile
                nc.vector.tensor_copy(dc[h][hi_row:hi_row + 64, :], dm[h][0:64, :])
                # lo part = ps - hi (fp16)
                nc.vector.tensor_sub(dc[h][lo_row:lo_row + 64, :], ps[psl, :],
                                     dm[h][0:64, :])
    for h in range(NH):
        nc.vector.memset(km[h][64:65, :], 1.0)

    # vp natural (t-major) + ones columns, bf16 (from fp16-hi inputs)
    vpx = [const.tile([128, DX], BF16, tag=f"vpx{t}", name=f"vpx{t}")
           for t in range(8)]
    for t in range(8):
        ps = ps_sm.tile([128, E], F32, tag="sm", name="pss")
        for e in range(4):
            nc.tensor.matmul(
                ps[:, 0:DX],
                lhsT=kvh_in[e][:, t * 128:(t + 1) * 128],
                rhs=wv[e][:],
                start=(e == 0), stop=(e == 3),
            )
        nc.scalar.copy(vpx[t][:], ps[:, 0:DX])
        for h4 in range(NH):
            c = h4 * (D + 1) + D
            nc.gpsimd.memset(vpx[t][:, c:c + 1], 1.0)

    # ---- per-head attention ----
    ypk = [const.tile([128, T], F32, tag=f"ypk{g}", name=f"ypk{g}")
           for g in range(2)]
    for h in range(NH):
        # max pass: S hi-only (fp16), row max per 128-row block
        colmax = const.tile([128, 8], F32, tag=f"cm{h}", name=f"cm{h}")
        for i in range(8):
            ps = ps_big.tile([128, T], F32, tag="big", name="psb")
            for n in range(2):
                nsl = slice(n * 512, (n + 1) * 512)
                nc.tensor.matmul(
                    ps[:, nsl],
                    lhsT=qm[h][0:64, i * 128:(i + 1) * 128],
                    rhs=km[h][0:64, nsl],
                    start=True, stop=True,
                )
            nc.vector.reduce_max(colmax[:, i:i + 1], ps[:], axis=AX.X,
                                 negate=True)
        # (128, 8) f32 -> (1, 1024) -> fp16 row 64 of qm, via DRAM bounce
        sc = dramp.tile([8, 128], F32, tag=f"sc{h}", name=f"sc{h}")
        nc.sync.dma_start(sc[:].rearrange("c p -> p c"), colmax[:])
        mxf = const.tile([1, T], F32, tag=f"mx{h}", name=f"mx{h}")
        nc.sync.dma_start(mxf[:], sc[:].rearrange("c p -> (c p)"))
        nc.scalar.copy(qm[h][64:65, :], mxf[:])

        # main pass: S^T - max = cross + main(bias), exp, AV accumulate
        oex = ps_av.tile([65, T], F32, tag="av", name="oex")
        for j in range(8):
            jsl = slice(j * 128, (j + 1) * 128)
            ps = ps_big.tile([128, T], F32, tag="big", name="psb")
            for n in range(2):
                nsl = slice(n * 512, (n + 1) * 512)
                nc.tensor.matmul(
                    ps[:, nsl], lhsT=kc[h][:, jsl], rhs=qc[h][:, nsl],
                    start=True, stop=False,
                )
                nc.tensor.matmul(
                    ps[:, nsl], lhsT=km[h][:, jsl], rhs=qm[h][:, nsl],
                    start=False, stop=True,
                )
            at = atp.tile([128, T], BF16, tag="at", name="at")
            nc.scalar.activation(at[:], ps[:], AF.Exp)
            for n in range(2):
                nsl = slice(n * 512, (n + 1) * 512)
                nc.tensor.matmul(
                    oex[:, nsl],
                    lhsT=vpx[j][:, h * (D + 1):(h + 1) * (D + 1)],
                    rhs=at[:, nsl],
                    start=(j == 0), stop=(j == 7),
                )

        # normalize: Y^T = oex[0:64] / rowsums (oex row 64)
        recip = const.tile([1, T], F32, tag=f"rcp{h}", name=f"rcp{h}")
        nc.vector.reciprocal(recip[:], oex[64:65, :])
        recb = const.tile([64, T], F32, tag=f"rcb{h}", name=f"rcb{h}")
        nc.gpsimd.partition_broadcast(recb[:], recip[:])
        g, half = divmod(h, 2)
        nc.vector.tensor_mul(
            ypk[g][half * 64:(half + 1) * 64, :], oex[0:64, :], recb[:]
        )

    # ---- output projection (fp16 single: Y/Wo errors stay relative) ----
    yh = [const.tile([128, T], F16, tag=f"yh{g}", name=f"yh{g}") for g in range(2)]
    for g in range(2):
        nc.scalar.copy(yh[g][:], ypk[g][:])
    for i in range(8):
        pso = ps_sm.tile([128, E], F32, tag="sm", name="pso")
        for g in range(2):
            nc.tensor.matmul(
                pso[:],
                lhsT=yh[g][:, i * 128:(i + 1) * 128],
                rhs=wo[g][:],
                start=(g == 0), stop=(g == 1),
            )
        ot = outp.tile([128, E], F32, tag="ot", name="ot")
        nc.vector.tensor_copy(ot[:], pso[:])
        nc.sync.dma_start(dram["out"][i * 128:(i + 1) * 128, :], ot[:])


def build_program():
    # Bacc (not raw Bass): its compile() splits multi-sem matmul waits onto
    # ldweights (TRN2 allows 1 wait/instruction), auto-inserts gpsimd
    # library loads for PartitionBroadcast, and lowers extended-ISA bytes.
    nc = bacc.Bacc("TRN2", target_bir_lowering=False, debug=False)
    dp = nc.declare_dram_parameter
    dram = {}
    for name in ("qh", "ql", "kvh", "kvl"):
        dram[name] = dp(name, [E, T], F16, isOutput=False)
    for name in ("wqh", "wql", "wkh", "wkl"):
        dram[name] = dp(name, [E, NH * D], F16, isOutput=False)
    dram["wv"] = dp("wv", [E, DX], F16, isOutput=False)
    dram["wo"] = dp("wo", [NH * D, E], F16, isOutput=False)
    dram["out"] = dp("out", [T, E], F32, isOutput=True)
    with ExitStack() as ctx:
        tc = ctx.enter_context(tile.TileContext(nc))
        _emit(ctx, tc, dram)
    nc.finalize()  # Bacc.finalize runs compile() then freezes
    return nc


_PROGRAM = None


def _get_program():
    global _PROGRAM
    if _PROGRAM is None:
        _PROGRAM = build_program()
    return _PROGRAM


def _split16(x):
    h = x.astype(np.float16)
    l = (x - h.astype(np.float32)).astype(np.float16)
    return h, l


def make_in_maps(q, kv, Wq, Wk, Wv, Wo):
    in_maps = []
    for c in range(N_CORES):
        b, g = divmod(c, 2)
        heads = [g * NH + j for j in range(NH)]
        idx_q = [d * H + h for h in heads for d in range(D)]
        idx_k = [((D - d) % D) * H + h for h in heads for d in range(D)]
        qTh, qTl = _split16(np.ascontiguousarray(q[b].T))
        kvTh, kvTl = _split16(np.ascontiguousarray(kv[b].T))
        wq_h, wq_l = _split16(Wq[:, idx_q] * np.float32(SCALE))
        wk_h, wk_l = _split16(Wk[:, idx_k])
        wv_c = np.zeros((E, DX), np.float16)
        for j, h in enumerate(heads):
            wv_c[:, j * (D + 1):j * (D + 1) + D] = \
                Wv[:, [d * H + h for d in range(D)]].astype(np.float16)
        in_maps.append({
            "qh": qTh, "ql": qTl, "kvh": kvTh, "kvl": kvTl,
            "wqh": wq_h, "wql": wq_l, "wkh": wk_h, "wkl": wk_l,
            "wv": wv_c,
            "wo": Wo[g * NH * D:(g + 1) * NH * D, :].astype(np.float16),
        })
    return in_maps


def kernel(**inputs):
    global LAST_EXEC_NS
    q = np.asarray(inputs["q"], dtype=np.float32)
    kv = np.asarray(inputs["kv"], dtype=np.float32)
    Wq = np.asarray(inputs["Wq"], dtype=np.float32)
    Wk = np.asarray(inputs["Wk"], dtype=np.float32)
    Wv = np.asarray(inputs["Wv"], dtype=np.float32)
    Wo = np.asarray(inputs["Wo"], dtype=np.float32)
    B = q.shape[0]

    nc = _get_program()
    in_maps = make_in_maps(q, kv, Wq, Wk, Wv, Wo)
    res = run_bass_kernel_spmd(nc, in_maps, list(range(N_CORES)), trace=TRACE)
    LAST_EXEC_NS = res.exec_time_ns

    out = np.empty((B, T, E), np.float32)
    for b in range(B):
        out[b] = res.results[2 * b]["out"] + res.results[2 * b + 1]["out"]
    return out


# revision 17
# speedup vs baseline: 1.7092x; 1.0368x over previous
"""Trainium2 Bass kernel for MultiHeadFrequencyCrossAttention.

Math note: the reference computes, per (batch, head) slice,
    energy = ifft2( fft2(Q) @ fft2(K)^T * dk ).real
Because the DFT matrix F satisfies F @ F^T = n * P (P = index-negation
permutation), this collapses EXACTLY to
    energy = dk * D * Q @ K~^T        with K~[j, d] = K[j, (-d) mod D]
i.e. plain attention with K's head-dim index flipped (mod D) and an extra
scale of dk * D = 512.  No FFTs are needed; the flip and scale are folded
into host-side slices of the Wk / Wq projection weights.

Sharding: 8 cores = 4 batches x 2 head-groups (4 heads each).  Each core
gets q[b]^T, kv[b]^T (pre-transposed on host so the contraction dim lands
on SBUF partitions) plus its slice of the projection weights, computes
attention for its 4 heads, and applies its slice of Wo.  The host sums the
two partial Wo products per batch (the unshard-reduce).

Precision scheme (PE fp32 matmuls are 4 cyc/row; fp16 is 1 cyc/row):
every value on the logit path is split hi/lo into two fp16 parts
(x = xh + xl, products of fp16 are exact in the fp32 PSUM accumulator), so
  x @ y ~= xh@yh + (xh@yl + xl@yh)     [~22-bit mantissa, err ~1e-6 rel]
One extra all-ones row in the stationary K operand times a "-rowmax" row
in the moving Q operand injects the softmax max-subtraction bias directly
into the S^T matmul.  The row max itself comes from a separate hi-only
fp16 pass (error ~ +-15 absolute on ~25000-scale logits, well inside the
exp() range window since A tiles are bf16).  A/V/output paths are plain
16-bit (error there stays relative, ~2e-3, no sharp-softmax blowup).

Per-core pipeline (T=1024, D=64):
  1. hi/lo projections -> per head: qm/km (fp16 hi + bias/ones row),
     qc/kc (fp16 [lo;hi] stacks for the cross matmul); vp t-major bf16
     with an all-ones column per head.
  2. max pass: S = qh @ kh^T per 128-row block (fp16), DVE reduce_max
     (negated) -> DRAM bounce -> fp16 "-rowmax" row of qm.
  3. main pass: S^T - max = cross(K=128) + main(K=65, w/ bias row)
     matmuls, ACT exp psum->sbuf bf16 directly in (j, i) layout.
  4. AV: A^T tiles are the moving operand; vp (with ones column) is
     stationary, accumulating [Y^T; rowsums] in one psum tensor.
  5. normalize Y^T by 1/rowsums (partition_broadcast + DVE mul), cast
     fp16, Wo partial product (fp16), DMA out.
"""

import numpy as np
from contextlib import ExitStack

import concourse.bass as bass
import concourse.tile as tile
from concourse import bacc, mybir
from concourse.bass_utils import run_bass_kernel_spmd

F32 = mybir.dt.float32
F16 = mybir.dt.float16
BF16 = mybir.dt.bfloat16
AX = mybir.AxisListType
AF = mybir.ActivationFunctionType

T = 1024          # sequence length
E = 512           # embed dim
H = 8             # total heads
D = E // H        # head dim = 64
NH = 4            # heads per core
DX = NH * (D + 1) # vp columns incl. ones = 260
N_CORES = 8
SCALE = float(D) * float(D) ** 0.5  # dk * D = 512.0

TRACE = False          # set by test harness; adds NTFF profiling
LAST_EXEC_NS = None


def _emit(ctx, tc, dram):
    nc = tc.nc
    const = ctx.enter_context(tc.tile_pool(name="const", bufs=1))
    ps_big = ctx.enter_context(tc.tile_pool(name="ps_big", bufs=2, space="PSUM"))
    ps_av = ctx.enter_context(tc.tile_pool(name="ps_av", bufs=1, space="PSUM"))
    ps_sm = ctx.enter_context(tc.tile_pool(name="ps_sm", bufs=2, space="PSUM"))
    atp = ctx.enter_context(tc.tile_pool(name="atp", bufs=4))
    outp = ctx.enter_context(tc.tile_pool(name="outp", bufs=3))
    dramp = ctx.enter_context(tc.tile_pool(name="dramp", bufs=1, space="DRAM"))

    # ---- input loads (all fp16 on the wire) ----
    def load4(name, cols):
        ts = [const.tile([128, cols], F16, tag=f"{name}{e}", name=f"{name}{e}")
              for e in range(4)]
        for e in range(4):
            nc.sync.dma_start(ts[e][:], dram[name][e * 128:(e + 1) * 128, :])
        return ts

    # load order matters: the first projection matmuls need wqh+ql first
    wqh = load4("wqh", NH * D)
    ql_in = load4("ql", T)
    wql = load4("wql", NH * D)
    qh_in = load4("qh", T)
    wkh = load4("wkh", NH * D)
    kvl_in = load4("kvl", T)
    wkl = load4("wkl", NH * D)
    kvh_in = load4("kvh", T)
    wv = load4("wv", DX)
    wo = [const.tile([128, E], F16, tag=f"wo{g}", name=f"wo{g}") for g in range(2)]
    for g in range(2):
        nc.sync.dma_start(wo[g][:], dram["wo"][g * 128:(g + 1) * 128, :])

    # ---- hi/lo projections ----
    # per head: qm (65, T) fp16 = [qp_hi; -rowmax(fp16) later]
    #           km (65, T) fp16 = [kp_hi; ones]
    #           qc (128, T) fp16 = [qp_lo; qp_hi]   (cross moving operand)
    #           kc (128, T) fp16 = [kp_hi; kp_lo]   (cross stationary)
    qm = [const.tile([65, T], F16, tag=f"qm{h}", name=f"qm{h}") for h in range(NH)]
    km = [const.tile([65, T], F16, tag=f"km{h}", name=f"km{h}") for h in range(NH)]
    qc = [const.tile([128, T], F16, tag=f"qc{h}", name=f"qc{h}") for h in range(NH)]
    kc = [const.tile([128, T], F16, tag=f"kc{h}", name=f"kc{h}") for h in range(NH)]

    for wh, wl, xh, xl, dm, dc, hi_row in (
        (wqh, wql, qh_in, ql_in, qm, qc, 64),   # qc rows: [lo; hi]
        (wkh, wkl, kvh_in, kvl_in, km, kc, 0),  # kc rows: [hi; lo]
    ):
        for m in range(2):  # head pair
            msl = slice(m * 128, (m + 1) * 128)
            ps = ps_big.tile([128, T], F32, tag="big", name="psb")
            for n in range(2):
                nsl = slice(n * 512, (n + 1) * 512)
                mms = (
                    # cross: Wh @ xl  +  Wl @ xh
                    [(wh[e], xl[e]) for e in range(4)]
                    + [(wl[e], xh[e]) for e in range(4)]
                    # main: Wh @ xh
                    + [(wh[e], xh[e]) for e in range(4)]
                )
                for i_mm, (lw, rx) in enumerate(mms):
                    nc.tensor.matmul(
                        ps[:, nsl],
                        lhsT=lw[:, msl],
                        rhs=rx[:, nsl],
                        start=(i_mm == 0), stop=(i_mm == len(mms) - 1),
                    )
            for hh in range(2):
                h = 2 * m + hh
                psl = slice(hh * 64, hh * 64 + 64)
                lo_row = 64 - hi_row
                # hi part (fp16 cast) into the K=65 "main" tile
                nc.scalar.copy(dm[h][0:64, :], ps[psl, :])
                # hi copy into the cross tile
                nc.vector.tensor_copy(dc[h][hi_row:hi_row + 64, :], dm[h][0:64, :])
                # lo part = ps - hi (fp16)
                nc.vector.tensor_sub(dc[h][lo_row:lo_row + 64, :], ps[psl, :],
                                     dm[h][0:64, :])
    for h in range(NH):
        nc.vector.memset(km[h][64:65, :], 1.0)

    # vp natural (t-major) + ones columns, bf16 (from fp16-hi inputs)
    vpx = [const.tile([128, DX], BF16, tag=f"vpx{t}", name=f"vpx{t}")
           for t in range(8)]
    for t in range(8):
        ps = ps_sm.tile([128, E], F32, tag="sm", name="pss")
        for e in range(4):
            nc.tensor.matmul(
                ps[:, 0:DX],
                lhsT=kvh_in[e][:, t * 128:(t + 1) * 128],
                rhs=wv[e][:],
                start=(e == 0), stop=(e == 3),
            )
        nc.scalar.copy(vpx[t][:], ps[:, 0:DX])
        for h4 in range(NH):
            c = h4 * (D + 1) + D
            nc.gpsimd.memset(vpx[t][:, c:c + 1], 1.0)

    # ---- per-head attention ----
    # Phase 1: all max passes first.  The (128,8)->(1,1024) bounce and the
    # fp16 cast of the "-rowmax" row for head h then overlap with head
    # h+1's max matmuls (and later heads' main passes), keeping PE dense
    # (HAM stays warm) instead of stalling on the bounce every head.
    ypk = [const.tile([128, T], F32, tag=f"ypk{g}", name=f"ypk{g}")
           for g in range(2)]
    for h in range(NH):
        # max pass: S hi-only (fp16), row max per 128-row block
        colmax = const.tile([128, 8], F32, tag=f"cm{h}", name=f"cm{h}")
        for i in range(8):
            ps = ps_big.tile([128, T], F32, tag="big", name="psb")
            for n in range(2):
                nsl = slice(n * 512, (n + 1) * 512)
                nc.tensor.matmul(
                    ps[:, nsl],
                    lhsT=qm[h][0:64, i * 128:(i + 1) * 128],
                    rhs=km[h][0:64, nsl],
                    start=True, stop=True,
                )
            nc.vector.reduce_max(colmax[:, i:i + 1], ps[:], axis=AX.X,
                                 negate=True)
        # (128, 8) f32 -> (1, 1024) f32 row, via DRAM bounce
        sc = dramp.tile([8, 128], F32, tag=f"sc{h}", name=f"sc{h}")
        nc.sync.dma_start(sc[:].rearrange("c p -> p c"), colmax[:])
        mxf = const.tile([1, T], F32, tag=f"mx{h}", name=f"mx{h}")
        nc.sync.dma_start(mxf[:], sc[:].rearrange("c p -> (c p)"))
        nc.scalar.copy(qm[h][64:65, :], mxf[:])

    # Phase 2: main passes
    for h in range(NH):
        # main pass: S^T - max = cross + main(bias), exp, AV accumulate
        oex = ps_av.tile([65, T], F32, tag="av", name="oex")
        for j in range(8):
            jsl = slice(j * 128, (j + 1) * 128)
            ps = ps_big.tile([128, T], F32, tag="big", name="psb")
            for n in range(2):
                nsl = slice(n * 512, (n + 1) * 512)
                nc.tensor.matmul(
                    ps[:, nsl], lhsT=kc[h][:, jsl], rhs=qc[h][:, nsl],
                    start=True, stop=False,
                )
                nc.tensor.matmul(
                    ps[:, nsl], lhsT=km[h][:, jsl], rhs=qm[h][:, nsl],
                    start=False, stop=True,
                )
            at = atp.tile([128, T], BF16, tag="at", name="at")
            nc.scalar.activation(at[:], ps[:], AF.Exp)
            for n in range(2):
                nsl = slice(n * 512, (n + 1) * 512)
                nc.tensor.matmul(
                    oex[:, nsl],
                    lhsT=vpx[j][:, h * (D + 1):(h + 1) * (D + 1)],
                    rhs=at[:, nsl],
                    start=(j == 0), stop=(j == 7),
                )

        # normalize: Y^T = oex[0:64] / rowsums (oex row 64).  Reciprocal is
        # ~8 cyc/elem on DVE, so run it in a (128, 8) layout (bounce the
        # row through a DMA reshape) instead of 1024 elems on one lane.
        sums = const.tile([1, T], F32, tag=f"sm{h}", name=f"sums{h}")
        nc.vector.tensor_copy(sums[:], oex[64:65, :])
        sd = dramp.tile([T], F32, tag=f"sd{h}", name=f"sd{h}")
        nc.sync.dma_start(sd[:], sums[:])
        s8 = const.tile([128, 8], F32, tag=f"s8{h}", name=f"s8{h}")
        nc.sync.dma_start(s8[:], sd[:].rearrange("(c p) -> p c", p=128))
        r8 = const.tile([128, 8], F32, tag=f"r8{h}", name=f"r8{h}")
        nc.vector.reciprocal(r8[:], s8[:])
        rd = dramp.tile([T], F32, tag=f"rd{h}", name=f"rd{h}")
        nc.sync.dma_start(rd[:].rearrange("(c p) -> p c", p=128), r8[:])
        recip = const.tile([1, T], F32, tag=f"rcp{h}", name=f"rcp{h}")
        nc.sync.dma_start(recip[:], rd[:])
        recb = const.tile([64, T], F32, tag=f"rcb{h}", name=f"rcb{h}")
        nc.gpsimd.partition_broadcast(recb[:], recip[:])
        g, half = divmod(h, 2)
        nc.vector.tensor_mul(
            ypk[g][half * 64:(half + 1) * 64, :], oex[0:64, :], recb[:]
        )

    # ---- output projection (fp16 single: Y/Wo errors stay relative) ----
    yh = [const.tile([128, T], F16, tag=f"yh{g}", name=f"yh{g}") for g in range(2)]
    for g in range(2):
        nc.scalar.copy(yh[g][:], ypk[g][:])
    for i in range(8):
        pso = ps_sm.tile([128, E], F32, tag="sm", name="pso")
        for g in range(2):
            nc.tensor.matmul(
                pso[:],
                lhsT=yh[g][:, i * 128:(i + 1) * 128],
                rhs=wo[g][:],
                start=(g == 0), stop=(g == 1),
            )
        ot = outp.tile([128, E], F32, tag="ot", name="ot")
        nc.vector.tensor_copy(ot[:], pso[:])
        nc.sync.dma_start(dram["out"][i * 128:(i + 1) * 128, :], ot[:])


def build_program():
    # Bacc (not raw Bass): its compile() splits multi-sem matmul waits onto
    # ldweights (TRN2 allows 1 wait/instruction), auto-inserts gpsimd
    # library loads for PartitionBroadcast, and lowers extended-ISA bytes.
    nc = bacc.Bacc("TRN2", target_bir_lowering=False, debug=False)
    dp = nc.declare_dram_parameter
    dram = {}
    for name in ("qh", "ql", "kvh", "kvl"):
        dram[name] = dp(name, [E, T], F16, isOutput=False)
    for name in ("wqh", "wql", "wkh", "wkl"):
        dram[name] = dp(name, [E, NH * D], F16, isOutput=False)
    dram["wv"] = dp("wv", [E, DX], F16, isOutput=False)
    dram["wo"] = dp("wo", [NH * D, E], F16, isOutput=False)
    dram["out"] = dp("out", [T, E], F32, isOutput=True)
    with ExitStack() as ctx:
        tc = ctx.enter_context(tile.TileContext(nc))
        _emit(ctx, tc, dram)
    nc.finalize()  # Bacc.finalize runs compile() then freezes
    return nc


_PROGRAM = None


def _get_program():
    global _PROGRAM
    if _PROGRAM is None:
        _PROGRAM = build_program()
    return _PROGRAM


def _split16(x):
    h = x.astype(np.float16)
    l = (x - h.astype(np.float32)).astype(np.float16)
    return h, l


def make_in_maps(q, kv, Wq, Wk, Wv, Wo):
    in_maps = []
    for c in range(N_CORES):
        b, g = divmod(c, 2)
        heads = [g * NH + j for j in range(NH)]
        idx_q = [d * H + h for h in heads for d in range(D)]
        idx_k = [((D - d) % D) * H + h for h in heads for d in range(D)]
        qTh, qTl = _split16(np.ascontiguousarray(q[b].T))
        kvTh, kvTl = _split16(np.ascontiguousarray(kv[b].T))
        wq_h, wq_l = _split16(Wq[:, idx_q] * np.float32(SCALE))
        wk_h, wk_l = _split16(Wk[:, idx_k])
        wv_c = np.zeros((E, DX), np.float16)
        for j, h in enumerate(heads):
            wv_c[:, j * (D + 1):j * (D + 1) + D] = \
                Wv[:, [d * H + h for d in range(D)]].astype(np.float16)
        in_maps.append({
            "qh": qTh, "ql": qTl, "kvh": kvTh, "kvl": kvTl,
            "wqh": wq_h, "wql": wq_l, "wkh": wk_h, "wkl": wk_l,
            "wv": wv_c,
            "wo": Wo[g * NH * D:(g + 1) * NH * D, :].astype(np.float16),
        })
    return in_maps


def kernel(**inputs):
    global LAST_EXEC_NS
    q = np.asarray(inputs["q"], dtype=np.float32)
    kv = np.asarray(inputs["kv"], dtype=np.float32)
    Wq = np.asarray(inputs["Wq"], dtype=np.float32)
    Wk = np.asarray(inputs["Wk"], dtype=np.float32)
    Wv = np.asarray(inputs["Wv"], dtype=np.float32)
    Wo = np.asarray(inputs["Wo"], dtype=np.float32)
    B = q.shape[0]

    nc = _get_program()
    in_maps = make_in_maps(q, kv, Wq, Wk, Wv, Wo)
    res = run_bass_kernel_spmd(nc, in_maps, list(range(N_CORES)), trace=TRACE)
    LAST_EXEC_NS = res.exec_time_ns

    out = np.empty((B, T, E), np.float32)
    for b in range(B):
        out[b] = res.results[2 * b]["out"] + res.results[2 * b + 1]["out"]
    return out


# revision 19
# speedup vs baseline: 1.8548x; 1.0852x over previous
"""Trainium2 Bass kernel for MultiHeadFrequencyCrossAttention.

Math note: the reference computes, per (batch, head) slice,
    energy = ifft2( fft2(Q) @ fft2(K)^T * dk ).real
Because the DFT matrix F satisfies F @ F^T = n * P (P = index-negation
permutation), this collapses EXACTLY to
    energy = dk * D * Q @ K~^T        with K~[j, d] = K[j, (-d) mod D]
i.e. plain attention with K's head-dim index flipped (mod D) and an extra
scale of dk * D = 512.  No FFTs are needed; the flip and scale are folded
into host-side slices of the Wk / Wq projection weights.

Sharding: 8 cores = 4 batches x 2 head-groups (4 heads each).  Each core
gets q[b]^T, kv[b]^T (pre-transposed on host so the contraction dim lands
on SBUF partitions) plus its slice of the projection weights, computes
attention for its 4 heads, and applies its slice of Wo.  The host sums the
two partial Wo products per batch (the unshard-reduce).

Precision scheme (PE fp32 matmuls are 4 cyc/row; fp16 is 1 cyc/row):
every value on the logit path is split hi/lo into two fp16 parts
(x = xh + xl, products of fp16 are exact in the fp32 PSUM accumulator), so
  x @ y ~= xh@yh + (xh@yl + xl@yh)     [~22-bit mantissa, err ~1e-6 rel]
One extra all-ones row in the stationary K operand times a "-rowmax" row
in the moving Q operand injects the softmax max-subtraction bias directly
into the S^T matmul.  The row max itself comes from a separate hi-only
fp16 pass (error ~ +-15 absolute on ~25000-scale logits, well inside the
exp() range window since A tiles are bf16).  A/V/output paths are plain
16-bit (error there stays relative, ~2e-3, no sharp-softmax blowup).

Per-core pipeline (T=1024, D=64):
  1. hi/lo projections -> per head: qm/km (fp16 hi + bias/ones row),
     qc/kc (fp16 [lo;hi] stacks for the cross matmul); vp t-major bf16
     with an all-ones column per head.
  2. max pass: S = qh @ kh^T per 128-row block (fp16), DVE reduce_max
     (negated) -> DRAM bounce -> fp16 "-rowmax" row of qm.
  3. main pass: S^T - max = cross(K=128) + main(K=65, w/ bias row)
     matmuls, ACT exp psum->sbuf bf16 directly in (j, i) layout.
  4. AV: A^T tiles are the moving operand; vp (with ones column) is
     stationary, accumulating [Y^T; rowsums] in one psum tensor.
  5. normalize Y^T by 1/rowsums (partition_broadcast + DVE mul), cast
     fp16, Wo partial product (fp16), DMA out.
"""

import numpy as np
from contextlib import ExitStack

import concourse.bass as bass
import concourse.tile as tile
from concourse import bacc, mybir
from concourse.bass_utils import run_bass_kernel_spmd

F32 = mybir.dt.float32
F16 = mybir.dt.float16
BF16 = mybir.dt.bfloat16
AX = mybir.AxisListType
AF = mybir.ActivationFunctionType

T = 1024          # sequence length
E = 512           # embed dim
H = 8             # total heads
D = E // H        # head dim = 64
NH = 4            # heads per core
DX = NH * (D + 1) # vp columns incl. ones = 260
N_CORES = 8
SCALE = float(D) * float(D) ** 0.5  # dk * D = 512.0

TRACE = False          # set by test harness; adds NTFF profiling
LAST_EXEC_NS = None


def _emit(ctx, tc, dram):
    nc = tc.nc
    const = ctx.enter_context(tc.tile_pool(name="const", bufs=1))
    ps_big = ctx.enter_context(tc.tile_pool(name="ps_big", bufs=2, space="PSUM"))
    ps_av = ctx.enter_context(tc.tile_pool(name="ps_av", bufs=1, space="PSUM"))
    ps_sm = ctx.enter_context(tc.tile_pool(name="ps_sm", bufs=2, space="PSUM"))
    atp = ctx.enter_context(tc.tile_pool(name="atp", bufs=4))
    outp = ctx.enter_context(tc.tile_pool(name="outp", bufs=3))
    dramp = ctx.enter_context(tc.tile_pool(name="dramp", bufs=1, space="DRAM"))

    # ---- input loads (all fp16 on the wire, one 3D DMA per matrix) ----
    def load1(name, cols):
        t3 = const.tile([128, 4, cols], F16, tag=name, name=name)
        nc.sync.dma_start(
            t3[:], dram[name][:].rearrange("(c p) t -> p c t", p=128)
        )
        return [t3[:, e, :] for e in range(4)]

    # load order matters: the first projection matmuls need wqh+ql first
    wqh = load1("wqh", NH * D)
    ql_in = load1("ql", T)
    wql = load1("wql", NH * D)
    qh_in = load1("qh", T)
    wkh = load1("wkh", NH * D)
    kvl_in = load1("kvl", T)
    wkl = load1("wkl", NH * D)
    kvh_in = load1("kvh", T)
    wv = load1("wv", DX)
    wo3 = const.tile([128, 2, E], F16, tag="wo", name="wo")
    nc.sync.dma_start(
        wo3[:], dram["wo"][:].rearrange("(g p) t -> p g t", p=128)
    )
    wo = [wo3[:, g, :] for g in range(2)]

    # ---- hi/lo projections ----
    # per head: qm (65, T) fp16 = [qp_hi; -rowmax(fp16) later]
    #           km (65, T) fp16 = [kp_hi; ones]
    #           qc (128, T) fp16 = [qp_lo; qp_hi]   (cross moving operand)
    #           kc (128, T) fp16 = [kp_hi; kp_lo]   (cross stationary)
    qm = [const.tile([65, T], F16, tag=f"qm{h}", name=f"qm{h}") for h in range(NH)]
    km = [const.tile([65, T], F16, tag=f"km{h}", name=f"km{h}") for h in range(NH)]
    qc = [const.tile([128, T], F16, tag=f"qc{h}", name=f"qc{h}") for h in range(NH)]
    kc = [const.tile([128, T], F16, tag=f"kc{h}", name=f"kc{h}") for h in range(NH)]

    for wh, wl, xh, xl, dm, dc, hi_row in (
        (wqh, wql, qh_in, ql_in, qm, qc, 64),   # qc rows: [lo; hi]
        (wkh, wkl, kvh_in, kvl_in, km, kc, 0),  # kc rows: [hi; lo]
    ):
        for m in range(2):  # head pair
            msl = slice(m * 128, (m + 1) * 128)
            ps = ps_big.tile([128, T], F32, tag="big", name="psb")
            for n in range(2):
                nsl = slice(n * 512, (n + 1) * 512)
                mms = (
                    # cross: Wh @ xl  +  Wl @ xh
                    [(wh[e], xl[e]) for e in range(4)]
                    + [(wl[e], xh[e]) for e in range(4)]
                    # main: Wh @ xh
                    + [(wh[e], xh[e]) for e in range(4)]
                )
                for i_mm, (lw, rx) in enumerate(mms):
                    nc.tensor.matmul(
                        ps[:, nsl],
                        lhsT=lw[:, msl],
                        rhs=rx[:, nsl],
                        start=(i_mm == 0), stop=(i_mm == len(mms) - 1),
                    )
            for hh in range(2):
                h = 2 * m + hh
                psl = slice(hh * 64, hh * 64 + 64)
                lo_row = 64 - hi_row
                # hi part (fp16 cast) into the K=65 "main" tile
                nc.scalar.copy(dm[h][0:64, :], ps[psl, :])
                # hi copy into the cross tile
                nc.vector.tensor_copy(dc[h][hi_row:hi_row + 64, :], dm[h][0:64, :])
                # lo part = ps - hi (fp16)
                nc.vector.tensor_sub(dc[h][lo_row:lo_row + 64, :], ps[psl, :],
                                     dm[h][0:64, :])
    for h in range(NH):
        nc.vector.memset(km[h][64:65, :], 1.0)

    # vp natural (t-major) + ones columns, bf16 (from fp16-hi inputs)
    vpx = [const.tile([128, DX], BF16, tag=f"vpx{t}", name=f"vpx{t}")
           for t in range(8)]
    for t in range(8):
        ps = ps_sm.tile([128, E], F32, tag="sm", name="pss")
        for e in range(4):
            nc.tensor.matmul(
                ps[:, 0:DX],
                lhsT=kvh_in[e][:, t * 128:(t + 1) * 128],
                rhs=wv[e][:],
                start=(e == 0), stop=(e == 3),
            )
        nc.scalar.copy(vpx[t][:], ps[:, 0:DX])
        for h4 in range(NH):
            c = h4 * (D + 1) + D
            nc.gpsimd.memset(vpx[t][:, c:c + 1], 1.0)

    # ---- per-head attention ----
    # Emission order software-pipelines heads: maxpass(0), maxpass(1),
    # main(0), maxpass(2), main(1), ... so the max-row DMA bounce and the
    # DVE reduce_max stream of head h+1 overlap head h's main-pass
    # matmuls, and PE never idles long enough to re-throttle (HAM).
    ypk = [const.tile([128, T], F32, tag=f"ypk{g}", name=f"ypk{g}")
           for g in range(2)]
    yun = [const.tile([64, T], F32, tag=f"yun{h}", name=f"yun{h}")
           for h in range(NH)]

    def maxpass(h):
        # max pass: S hi-only (fp16), row max per 128-row block
        colmax = const.tile([128, 8], F32, tag=f"cm{h}", name=f"cm{h}")
        for i in range(8):
            ps = ps_big.tile([128, T], F32, tag="big", name="psb")
            for n in range(2):
                nsl = slice(n * 512, (n + 1) * 512)
                nc.tensor.matmul(
                    ps[:, nsl],
                    lhsT=qm[h][0:64, i * 128:(i + 1) * 128],
                    rhs=km[h][0:64, nsl],
                    start=True, stop=True,
                )
            nc.vector.reduce_max(colmax[:, i:i + 1], ps[:], axis=AX.X,
                                 negate=True)
        # (128, 8) f32 -> (1, 1024) f32 row, via DRAM bounce
        sc = dramp.tile([8, 128], F32, tag=f"sc{h}", name=f"sc{h}")
        nc.sync.dma_start(sc[:].rearrange("c p -> p c"), colmax[:])
        mxf = const.tile([1, T], F32, tag=f"mx{h}", name=f"mx{h}")
        nc.sync.dma_start(mxf[:], sc[:].rearrange("c p -> (c p)"))
        nc.scalar.copy(qm[h][64:65, :], mxf[:])

    def mainpass(h):
        # main pass: S^T - max = cross + main(bias), exp, AV accumulate
        oex = ps_av.tile([65, T], F32, tag="av", name="oex")
        for j in range(8):
            jsl = slice(j * 128, (j + 1) * 128)
            ps = ps_big.tile([128, T], F32, tag="big", name="psb")
            for n in range(2):
                nsl = slice(n * 512, (n + 1) * 512)
                nc.tensor.matmul(
                    ps[:, nsl], lhsT=kc[h][:, jsl], rhs=qc[h][:, nsl],
                    start=True, stop=False,
                )
                nc.tensor.matmul(
                    ps[:, nsl], lhsT=km[h][:, jsl], rhs=qm[h][:, nsl],
                    start=False, stop=True,
                )
            at = atp.tile([128, T], BF16, tag="at", name="at")
            nc.scalar.activation(at[:], ps[:], AF.Exp)
            for n in range(2):
                nsl = slice(n * 512, (n + 1) * 512)
                nc.tensor.matmul(
                    oex[:, nsl],
                    lhsT=vpx[j][:, h * (D + 1):(h + 1) * (D + 1)],
                    rhs=at[:, nsl],
                    start=(j == 0), stop=(j == 7),
                )
        # Evacuate PSUM immediately (frees the oex slot for the next head);
        # the slow normalize chain below then runs off the critical path.
        nc.scalar.copy(yun[h][:], oex[0:64, :])
        sums = const.tile([1, T], F32, tag=f"sm{h}", name=f"sums{h}")
        nc.vector.tensor_copy(sums[:], oex[64:65, :])
        # reciprocal is ~8 cyc/elem on DVE: run it in a (128, 8) layout
        # (DMA reshape through DRAM) instead of 1024 elems on one lane
        sd = dramp.tile([T], F32, tag=f"sd{h}", name=f"sd{h}")
        nc.sync.dma_start(sd[:], sums[:])
        s8 = const.tile([128, 8], F32, tag=f"s8{h}", name=f"s8{h}")
        nc.sync.dma_start(s8[:], sd[:].rearrange("(c p) -> p c", p=128))
        r8 = const.tile([128, 8], F32, tag=f"r8{h}", name=f"r8{h}")
        nc.vector.reciprocal(r8[:], s8[:])
        rd = dramp.tile([T], F32, tag=f"rd{h}", name=f"rd{h}")
        nc.sync.dma_start(rd[:].rearrange("(c p) -> p c", p=128), r8[:])
        recip = const.tile([1, T], F32, tag=f"rcp{h}", name=f"rcp{h}")
        nc.sync.dma_start(recip[:], rd[:])
        recb = const.tile([64, T], F32, tag=f"rcb{h}", name=f"rcb{h}")
        nc.gpsimd.partition_broadcast(recb[:], recip[:])
        g, half = divmod(h, 2)
        nc.vector.tensor_mul(
            ypk[g][half * 64:(half + 1) * 64, :], yun[h][:], recb[:]
        )

    maxpass(0)
    maxpass(1)
    mainpass(0)
    maxpass(2)
    mainpass(1)
    maxpass(3)
    mainpass(2)
    mainpass(3)

    # ---- output projection (fp16 single: Y/Wo errors stay relative) ----
    yh = [const.tile([128, T], F16, tag=f"yh{g}", name=f"yh{g}") for g in range(2)]
    for g in range(2):
        nc.scalar.copy(yh[g][:], ypk[g][:])
    for i in range(8):
        pso = ps_sm.tile([128, E], F32, tag="sm", name="pso")
        for g in range(2):
            nc.tensor.matmul(
                pso[:],
                lhsT=yh[g][:, i * 128:(i + 1) * 128],
                rhs=wo[g][:],
                start=(g == 0), stop=(g == 1),
            )
        ot = outp.tile([128, E], F32, tag="ot", name="ot")
        nc.vector.tensor_copy(ot[:], pso[:])
        nc.sync.dma_start(dram["out"][i * 128:(i + 1) * 128, :], ot[:])


def build_program():
    # Bacc (not raw Bass): its compile() splits multi-sem matmul waits onto
    # ldweights (TRN2 allows 1 wait/instruction), auto-inserts gpsimd
    # library loads for PartitionBroadcast, and lowers extended-ISA bytes.
    nc = bacc.Bacc("TRN2", target_bir_lowering=False, debug=False)
    dp = nc.declare_dram_parameter
    dram = {}
    for name in ("qh", "ql", "kvh", "kvl"):
        dram[name] = dp(name, [E, T], F16, isOutput=False)
    for name in ("wqh", "wql", "wkh", "wkl"):
        dram[name] = dp(name, [E, NH * D], F16, isOutput=False)
    dram["wv"] = dp("wv", [E, DX], F16, isOutput=False)
    dram["wo"] = dp("wo", [NH * D, E], F16, isOutput=False)
    dram["out"] = dp("out", [T, E], F32, isOutput=True)
    with ExitStack() as ctx:
        tc = ctx.enter_context(tile.TileContext(nc))
        _emit(ctx, tc, dram)
    nc.finalize()  # Bacc.finalize runs compile() then freezes
    return nc


_PROGRAM = None


def _get_program():
    global _PROGRAM
    if _PROGRAM is None:
        _PROGRAM = build_program()
    return _PROGRAM


def _split16(x):
    h = x.astype(np.float16)
    l = (x - h.astype(np.float32)).astype(np.float16)
    return h, l


def make_in_maps(q, kv, Wq, Wk, Wv, Wo):
    in_maps = []
    for c in range(N_CORES):
        b, g = divmod(c, 2)
        heads = [g * NH + j for j in range(NH)]
        idx_q = [d * H + h for h in heads for d in range(D)]
        idx_k = [((D - d) % D) * H + h for h in heads for d in range(D)]
        qTh, qTl = _split16(np.ascontiguousarray(q[b].T))
        kvTh, kvTl = _split16(np.ascontiguousarray(kv[b].T))
        wq_h, wq_l = _split16(Wq[:, idx_q] * np.float32(SCALE))
        wk_h, wk_l = _split16(Wk[:, idx_k])
        wv_c = np.zeros((E, DX), np.float16)
        for j, h in enumerate(heads):
            wv_c[:, j * (D + 1):j * (D + 1) + D] = \
                Wv[:, [d * H + h for d in range(D)]].astype(np.float16)
        in_maps.append({
            "qh": qTh, "ql": qTl, "kvh": kvTh, "kvl": kvTl,
            "wqh": wq_h, "wql": wq_l, "wkh": wk_h, "wkl": wk_l,
            "wv": wv_c,
            "wo": Wo[g * NH * D:(g + 1) * NH * D, :].astype(np.float16),
        })
    return in_maps


def kernel(**inputs):
    global LAST_EXEC_NS
    q = np.asarray(inputs["q"], dtype=np.float32)
    kv = np.asarray(inputs["kv"], dtype=np.float32)
    Wq = np.asarray(inputs["Wq"], dtype=np.float32)
    Wk = np.asarray(inputs["Wk"], dtype=np.float32)
    Wv = np.asarray(inputs["Wv"], dtype=np.float32)
    Wo = np.asarray(inputs["Wo"], dtype=np.float32)
    B = q.shape[0]

    nc = _get_program()
    in_maps = make_in_maps(q, kv, Wq, Wk, Wv, Wo)
    res = run_bass_kernel_spmd(nc, in_maps, list(range(N_CORES)), trace=TRACE)
    LAST_EXEC_NS = res.exec_time_ns

    out = np.empty((B, T, E), np.float32)
    for b in range(B):
        out[b] = res.results[2 * b]["out"] + res.results[2 * b + 1]["out"]
    return out


# revision 22
# speedup vs baseline: 2.1746x; 1.1724x over previous
"""Trainium2 Bass kernel for MultiHeadFrequencyCrossAttention.

Math note: the reference computes, per (batch, head) slice,
    energy = ifft2( fft2(Q) @ fft2(K)^T * dk ).real
Because the DFT matrix F satisfies F @ F^T = n * P (P = index-negation
permutation), this collapses EXACTLY to
    energy = dk * D * Q @ K~^T        with K~[j, d] = K[j, (-d) mod D]
i.e. plain attention with K's head-dim index flipped (mod D) and an extra
scale of dk * D = 512.  No FFTs are needed; the flip and scale are folded
into host-side slices of the Wk / Wq projection weights.

Sharding: 8 cores = 4 batches x 2 head-groups (4 heads each).  Each core
gets q[b]^T, kv[b]^T (pre-transposed on host so the contraction dim lands
on SBUF partitions) plus its slice of the projection weights, computes
attention for its 4 heads, and applies its slice of Wo.  The host sums the
two partial Wo products per batch (the unshard-reduce).

Precision scheme (PE fp32 matmuls are 4 cyc/row; fp16 is 1 cyc/row):
every value on the logit path is split hi/lo into two fp16 parts
(x = xh + xl, products of fp16 are exact in the fp32 PSUM accumulator), so
  x @ y ~= xh@yh + (xh@yl + xl@yh)     [~22-bit mantissa, err ~1e-6 rel]
One extra all-ones row in the stationary K operand times a "-rowmax" row
in the moving Q operand injects the softmax max-subtraction bias directly
into the S^T matmul.  The row max itself comes from a separate hi-only
fp16 pass (error ~ +-15 absolute on ~25000-scale logits, well inside the
exp() range window since A tiles are bf16).  A/V/output paths are plain
16-bit (error there stays relative, ~2e-3, no sharp-softmax blowup).

Per-core pipeline (T=1024, D=64):
  1. hi/lo projections -> per head: qm/km (fp16 hi + bias/ones row),
     qc/kc (fp16 [lo;hi] stacks for the cross matmul); vp t-major bf16
     with an all-ones column per head.
  2. max pass: S = qh @ kh^T per 128-row block (fp16), DVE reduce_max
     (negated) -> DRAM bounce -> fp16 "-rowmax" row of qm.
  3. main pass: S^T - max = cross(K=128) + main(K=65, w/ bias row)
     matmuls, ACT exp psum->sbuf bf16 directly in (j, i) layout.
  4. AV: A^T tiles are the moving operand; vp (with ones column) is
     stationary, accumulating [Y^T; rowsums] in one psum tensor.
  5. normalize Y^T by 1/rowsums (partition_broadcast + DVE mul), cast
     fp16, Wo partial product (fp16), DMA out.
"""

import numpy as np
from contextlib import ExitStack

import concourse.bass as bass
import concourse.tile as tile
from concourse import bacc, mybir
from concourse.bass_utils import run_bass_kernel_spmd

F32 = mybir.dt.float32
F16 = mybir.dt.float16
BF16 = mybir.dt.bfloat16
AX = mybir.AxisListType
AF = mybir.ActivationFunctionType

T = 1024          # sequence length
E = 512           # embed dim
H = 8             # total heads
D = E // H        # head dim = 64
NH = 4            # heads per core
DX = NH * (D + 1) # vp columns incl. ones = 260
N_CORES = 8
SCALE = float(D) * float(D) ** 0.5  # dk * D = 512.0

TRACE = False          # set by test harness; adds NTFF profiling
LAST_EXEC_NS = None


def _emit(ctx, tc, dram):
    nc = tc.nc
    const = ctx.enter_context(tc.tile_pool(name="const", bufs=1))
    ps_big = ctx.enter_context(tc.tile_pool(name="ps_big", bufs=2, space="PSUM"))
    ps_av = ctx.enter_context(tc.tile_pool(name="ps_av", bufs=1, space="PSUM"))
    ps_sm = ctx.enter_context(tc.tile_pool(name="ps_sm", bufs=2, space="PSUM"))
    atp = ctx.enter_context(tc.tile_pool(name="atp", bufs=6))
    outp = ctx.enter_context(tc.tile_pool(name="outp", bufs=8))
    dramp = ctx.enter_context(tc.tile_pool(name="dramp", bufs=1, space="DRAM"))

    # ---- input loads (all fp16 on the wire, one 3D DMA per matrix) ----
    def load1(name, cols):
        t3 = const.tile([128, 4, cols], F16, tag=name, name=name)
        nc.sync.dma_start(
            t3[:], dram[name][:].rearrange("(c p) t -> p c t", p=128)
        )
        return [t3[:, e, :] for e in range(4)]

    # load order matters: the first projection matmuls need wqh+ql first
    wqh = load1("wqh", NH * D)
    ql_in = load1("ql", T)
    wql = load1("wql", NH * D)
    qh_in = load1("qh", T)
    wkh = load1("wkh", NH * D)
    kvl_in = load1("kvl", T)
    wkl = load1("wkl", NH * D)
    kvh_in = load1("kvh", T)
    wv = load1("wv", DX)
    wo3 = const.tile([128, 2, E], F16, tag="wo", name="wo")
    nc.sync.dma_start(
        wo3[:], dram["wo"][:].rearrange("(g p) t -> p g t", p=128)
    )
    wo = [wo3[:, g, :] for g in range(2)]

    # PE warm-up: dummy matmuls fill the input-DMA window so the HAM clock
    # gate is already at 8/8 (2.4 GHz) when the projections start.
    wrm = const.tile([128, 512], F16, tag="wrm", name="wrm")
    nc.vector.memset(wrm[:], 0.0)
    for w in range(16):
        pw = ps_sm.tile([128, E], F32, tag="sm", name="psw")
        nc.tensor.matmul(pw[:], lhsT=wrm[:, 0:128], rhs=wrm[:],
                         start=True, stop=True)

    # ---- hi/lo projections ----
    # per head: qm (65, T) fp16 = [qp_hi; -rowmax(fp16) later]
    #           km (65, T) fp16 = [kp_hi; ones]
    #           qc (128, T) fp16 = [qp_lo; qp_hi]   (cross moving operand)
    #           kc (128, T) fp16 = [kp_hi; kp_lo]   (cross stationary)
    qm = [const.tile([65, T], F16, tag=f"qm{h}", name=f"qm{h}") for h in range(NH)]
    km = [const.tile([65, T], F16, tag=f"km{h}", name=f"km{h}") for h in range(NH)]
    qc = [const.tile([128, T], F16, tag=f"qc{h}", name=f"qc{h}") for h in range(NH)]
    kc = [const.tile([128, T], F16, tag=f"kc{h}", name=f"kc{h}") for h in range(NH)]

    for wh, wl, xh, xl, dm, dc, hi_row in (
        (wqh, wql, qh_in, ql_in, qm, qc, 64),   # qc rows: [lo; hi]
        (wkh, wkl, kvh_in, kvl_in, km, kc, 0),  # kc rows: [hi; lo]
    ):
        for m in range(2):  # head pair
            msl = slice(m * 128, (m + 1) * 128)
            ps = ps_big.tile([128, T], F32, tag="big", name="psb")
            for n in range(2):
                nsl = slice(n * 512, (n + 1) * 512)
                mms = (
                    # cross: Wh @ xl  +  Wl @ xh
                    [(wh[e], xl[e]) for e in range(4)]
                    + [(wl[e], xh[e]) for e in range(4)]
                    # main: Wh @ xh
                    + [(wh[e], xh[e]) for e in range(4)]
                )
                for i_mm, (lw, rx) in enumerate(mms):
                    nc.tensor.matmul(
                        ps[:, nsl],
                        lhsT=lw[:, msl],
                        rhs=rx[:, nsl],
                        start=(i_mm == 0), stop=(i_mm == len(mms) - 1),
                    )
            for hh in range(2):
                h = 2 * m + hh
                psl = slice(hh * 64, hh * 64 + 64)
                lo_row = 64 - hi_row
                # hi part (fp16 cast) into the K=65 "main" tile
                nc.scalar.copy(dm[h][0:64, :], ps[psl, :])
                # hi copy into the cross tile
                nc.vector.tensor_copy(dc[h][hi_row:hi_row + 64, :], dm[h][0:64, :])
                # lo part = ps - hi (fp16)
                nc.vector.tensor_sub(dc[h][lo_row:lo_row + 64, :], ps[psl, :],
                                     dm[h][0:64, :])
    for h in range(NH):
        nc.vector.memset(km[h][64:65, :], 1.0)

    # vp natural (t-major) + ones columns, bf16 (from fp16-hi inputs)
    vpx = [const.tile([128, DX], BF16, tag=f"vpx{t}", name=f"vpx{t}")
           for t in range(8)]
    for t in range(8):
        ps = ps_sm.tile([128, E], F32, tag="sm", name="pss")
        for e in range(4):
            nc.tensor.matmul(
                ps[:, 0:DX],
                lhsT=kvh_in[e][:, t * 128:(t + 1) * 128],
                rhs=wv[e][:],
                start=(e == 0), stop=(e == 3),
            )
        nc.scalar.copy(vpx[t][:], ps[:, 0:DX])
        for h4 in range(NH):
            c = h4 * (D + 1) + D
            nc.gpsimd.memset(vpx[t][:, c:c + 1], 1.0)

    # ---- per-head attention ----
    # Emission order software-pipelines heads: maxpass(0), maxpass(1),
    # main(0), maxpass(2), main(1), ... so the max-row DMA bounce and the
    # DVE reduce_max stream of head h+1 overlap head h's main-pass
    # matmuls, and PE never idles long enough to re-throttle (HAM).
    ypk = [const.tile([128, T], F32, tag=f"ypk{g}", name=f"ypk{g}")
           for g in range(2)]
    yun = [const.tile([64, T], F32, tag=f"yun{h}", name=f"yun{h}")
           for h in range(NH)]

    def maxpass(h):
        # max pass: S hi-only (fp16), row max per 128-row block
        colmax = const.tile([128, 8], F32, tag=f"cm{h}", name=f"cm{h}")
        for i in range(8):
            ps = ps_big.tile([128, T], F32, tag="big", name="psb")
            for n in range(2):
                nsl = slice(n * 512, (n + 1) * 512)
                nc.tensor.matmul(
                    ps[:, nsl],
                    lhsT=qm[h][0:64, i * 128:(i + 1) * 128],
                    rhs=km[h][0:64, nsl],
                    start=True, stop=True,
                )
            nc.vector.reduce_max(colmax[:, i:i + 1], ps[:], axis=AX.X,
                                 negate=True)
        # (128, 8) f32 -> (1, 1024) f32 row, via DRAM bounce
        sc = dramp.tile([8, 128], F32, tag=f"sc{h}", name=f"sc{h}")
        nc.sync.dma_start(sc[:].rearrange("c p -> p c"), colmax[:])
        mxf = const.tile([1, T], F32, tag=f"mx{h}", name=f"mx{h}")
        nc.sync.dma_start(mxf[:], sc[:].rearrange("c p -> (c p)"))
        nc.scalar.copy(qm[h][64:65, :], mxf[:])

    def mainpass(h):
        # main pass: S^T - max = cross + main(bias), exp, AV accumulate
        oex = ps_av.tile([65, T], F32, tag="av", name="oex")
        for j in range(8):
            jsl = slice(j * 128, (j + 1) * 128)
            ps = ps_big.tile([128, T], F32, tag="big", name="psb")
            for n in range(2):
                nsl = slice(n * 512, (n + 1) * 512)
                nc.tensor.matmul(
                    ps[:, nsl], lhsT=kc[h][:, jsl], rhs=qc[h][:, nsl],
                    start=True, stop=False,
                )
                nc.tensor.matmul(
                    ps[:, nsl], lhsT=km[h][:, jsl], rhs=qm[h][:, nsl],
                    start=False, stop=True,
                )
            at = atp.tile([128, T], BF16, tag="at", name="at")
            nc.scalar.activation(at[:], ps[:], AF.Exp)
            for n in range(2):
                nsl = slice(n * 512, (n + 1) * 512)
                nc.tensor.matmul(
                    oex[:, nsl],
                    lhsT=vpx[j][:, h * (D + 1):(h + 1) * (D + 1)],
                    rhs=at[:, nsl],
                    start=(j == 0), stop=(j == 7),
                )
        # Evacuate PSUM immediately (frees the oex slot for the next head);
        # the normalize chain below then runs off the critical path.
        nc.scalar.copy(yun[h][:], oex[0:64, :])
        # 1/sums via exp(-log(sums)) on ACT: Log+Exp live in one table set
        # and DVE reciprocal on a 1-partition row would be 8 cyc/elem.
        lns = const.tile([1, T], F32, tag=f"ln{h}", name=f"ln{h}")
        nc.scalar.activation(lns[:], oex[64:65, :], AF.Ln)
        recip = const.tile([1, T], F32, tag=f"rcp{h}", name=f"rcp{h}")
        nc.scalar.activation(recip[:], lns[:], AF.Exp, scale=-1.0)
        recb = const.tile([64, T], F32, tag=f"rcb{h}", name=f"rcb{h}")
        nc.gpsimd.partition_broadcast(recb[:], recip[:])
        g, half = divmod(h, 2)
        nc.vector.tensor_mul(
            ypk[g][half * 64:(half + 1) * 64, :], yun[h][:], recb[:]
        )

    # ---- output projection, split by head pair ----
    # g=0 (heads 0,1) runs mid-kernel right after ypk[0] completes; g=1
    # accumulates on top at the tail.  fp16 single: Y/Wo errors stay
    # relative (~5e-4), no sharp-softmax amplification.
    yh = [const.tile([128, T], F16, tag=f"yh{g}", name=f"yh{g}") for g in range(2)]
    ot = [outp.tile([128, E], F32, tag="ot", name=f"ot{i}") for i in range(8)]

    def wo_pass(g):
        nc.scalar.copy(yh[g][:], ypk[g][:])
        for i in range(8):
            pso = ps_sm.tile([128, E], F32, tag="sm", name="pso")
            nc.tensor.matmul(
                pso[:],
                lhsT=yh[g][:, i * 128:(i + 1) * 128],
                rhs=wo[g][:],
                start=True, stop=True,
            )
            if g == 0:
                nc.vector.tensor_copy(ot[i][:], pso[:])
            else:
                nc.vector.tensor_add(ot[i][:], ot[i][:], pso[:])
                nc.sync.dma_start(dram["out"][i * 128:(i + 1) * 128, :], ot[i][:])

    maxpass(0)
    maxpass(1)
    mainpass(0)
    maxpass(2)
    mainpass(1)
    maxpass(3)
    wo_pass(0)
    mainpass(2)
    mainpass(3)
    wo_pass(1)

def build_program():
    # Bacc (not raw Bass): its compile() splits multi-sem matmul waits onto
    # ldweights (TRN2 allows 1 wait/instruction), auto-inserts gpsimd
    # library loads for PartitionBroadcast, and lowers extended-ISA bytes.
    nc = bacc.Bacc("TRN2", target_bir_lowering=False, debug=False)
    dp = nc.declare_dram_parameter
    dram = {}
    for name in ("qh", "ql", "kvh", "kvl"):
        dram[name] = dp(name, [E, T], F16, isOutput=False)
    for name in ("wqh", "wql", "wkh", "wkl"):
        dram[name] = dp(name, [E, NH * D], F16, isOutput=False)
    dram["wv"] = dp("wv", [E, DX], F16, isOutput=False)
    dram["wo"] = dp("wo", [NH * D, E], F16, isOutput=False)
    dram["out"] = dp("out", [T, E], F32, isOutput=True)
    with ExitStack() as ctx:
        tc = ctx.enter_context(tile.TileContext(nc))
        _emit(ctx, tc, dram)
    nc.finalize()  # Bacc.finalize runs compile() then freezes
    return nc


_PROGRAM = None


def _get_program():
    global _PROGRAM
    if _PROGRAM is None:
        _PROGRAM = build_program()
    return _PROGRAM


def _split16(x):
    h = x.astype(np.float16)
    l = (x - h.astype(np.float32)).astype(np.float16)
    return h, l


def make_in_maps(q, kv, Wq, Wk, Wv, Wo):
    in_maps = []
    for c in range(N_CORES):
        b, g = divmod(c, 2)
        heads = [g * NH + j for j in range(NH)]
        idx_q = [d * H + h for h in heads for d in range(D)]
        idx_k = [((D - d) % D) * H + h for h in heads for d in range(D)]
        qTh, qTl = _split16(np.ascontiguousarray(q[b].T))
        kvTh, kvTl = _split16(np.ascontiguousarray(kv[b].T))
        wq_h, wq_l = _split16(Wq[:, idx_q] * np.float32(SCALE))
        wk_h, wk_l = _split16(Wk[:, idx_k])
        wv_c = np.zeros((E, DX), np.float16)
        for j, h in enumerate(heads):
            wv_c[:, j * (D + 1):j * (D + 1) + D] = \
                Wv[:, [d * H + h for d in range(D)]].astype(np.float16)
        in_maps.append({
            "qh": qTh, "ql": qTl, "kvh": kvTh, "kvl": kvTl,
            "wqh": wq_h, "wql": wq_l, "wkh": wk_h, "wkl": wk_l,
            "wv": wv_c,
            "wo": Wo[g * NH * D:(g + 1) * NH * D, :].astype(np.float16),
        })
    return in_maps


def kernel(**inputs):
    global LAST_EXEC_NS
    q = np.asarray(inputs["q"], dtype=np.float32)
    kv = np.asarray(inputs["kv"], dtype=np.float32)
    Wq = np.asarray(inputs["Wq"], dtype=np.float32)
    Wk = np.asarray(inputs["Wk"], dtype=np.float32)
    Wv = np.asarray(inputs["Wv"], dtype=np.float32)
    Wo = np.asarray(inputs["Wo"], dtype=np.float32)
    B = q.shape[0]

    nc = _get_program()
    in_maps = make_in_maps(q, kv, Wq, Wk, Wv, Wo)
    res = run_bass_kernel_spmd(nc, in_maps, list(range(N_CORES)), trace=TRACE)
    LAST_EXEC_NS = res.exec_time_ns

    out = np.empty((B, T, E), np.float32)
    for b in range(B):
        out[b] = res.results[2 * b]["out"] + res.results[2 * b + 1]["out"]
    return out
